# revision 8
# baseline (speedup 1.0000x reference)
"""Trainium2 Bass kernel for a 2-layer bidirectional RNN (tanh cells) + linear head.

Model (hardcoded shapes): B=128, S=512, E=512, H=1024, V=50000, C=2.

Sharding: 8 cores = 2 directions x 4 batch-quarters (B_local=32). Each core runs
both stacked layers of its direction over the full sequence, chunk by chunk:
  gather emb rows -> PE-transpose to e^T -> P0 = e @ Wi0 + biases (big matmul)
  -> layer-0 scan (h^T kept transposed; Wh streamed at N=512)
  -> P1 = h0 @ Wi1 + biases -> layer-1 scan -> final state -> head partial.
The two fc layers have no nonlinearity between them, so they collapse to a
single [2H, 2] matrix W12 = fc1_W @ fc2_W; each core computes its direction's
partial product on device and the host sums partials + b12 during unshard.
All matmuls run in bf16 with fp32 PSUM accumulation (validated: rel err ~3e-3).
"""

import numpy as np
import ml_dtypes

import concourse.bass as bass
import concourse.mybir as mybir
import concourse.tile as tile
from concourse import bacc
from concourse.masks import make_identity
from concourse.bass_utils import run_bass_kernel_spmd

F32 = mybir.dt.float32
BF16 = mybir.dt.bfloat16
I32 = mybir.dt.int32

B, S, E, H, V, C = 128, 512, 512, 1024, 50000, 2
BL = 32          # batch per core
KT = H // 128    # 8 k-tiles of the hidden dim
ET = E // 128    # 4 k-tiles of the embedding dim

_CACHE = {}


def _build(S_=S, T=16):
    """Build the (uniform, per-core) Bass program. T = steps per chunk."""
    NCH = S_ // T            # chunks
    GPC = (T * BL) // 128    # gather tiles per chunk (tb-tiles)
    NT = NCH * GPC           # total tb-tiles

    nc = bacc.Bacc(None, target_bir_lowering=False, debug=True)

    emb_d = nc.dram_tensor("emb", [V, E], F32, kind="ExternalInput")
    idx_d = nc.dram_tensor("idx", [128, NT], I32, kind="ExternalInput")
    wi0_d = nc.dram_tensor("wi0", [128, ET, H], BF16, kind="ExternalInput")
    wh0_d = nc.dram_tensor("wh0", [128, KT, H], BF16, kind="ExternalInput")
    wi1_d = nc.dram_tensor("wi1", [128, KT, H], BF16, kind="ExternalInput")
    wh1_d = nc.dram_tensor("wh1", [128, KT, H], BF16, kind="ExternalInput")
    b0_d = nc.dram_tensor("bias0", [1, H], BF16, kind="ExternalInput")
    b1_d = nc.dram_tensor("bias1", [1, H], BF16, kind="ExternalInput")
    w12_d = nc.dram_tensor("w12", [128, KT, C], F32, kind="ExternalInput")
    out_d = nc.dram_tensor("out", [BL, C], F32, kind="ExternalOutput")

    with tile.TileContext(nc) as tc:
        with (
            tc.tile_pool(name="const", bufs=1) as const,
            tc.tile_pool(name="gat", bufs=4) as gat,
            tc.tile_pool(name="et", bufs=2) as etp,
            tc.tile_pool(name="p0", bufs=2) as p0p,
            tc.tile_pool(name="p1", bufs=2) as p1p,
            tc.tile_pool(name="ch0", bufs=2) as ch0p,
            tc.tile_pool(name="st1", bufs=2) as st1p,
            tc.tile_pool(name="act", bufs=2) as actp,
            tc.tile_pool(name="osb", bufs=1) as osb,
            tc.tile_pool(name="sps", bufs=2, space="PSUM") as spsp,
            tc.tile_pool(name="trs", bufs=2, space="PSUM") as trsp,
            tc.tile_pool(name="pps", bufs=2, space="PSUM") as ppsp,
        ):
            # ---- constants / weights into SBUF ----
            idx_sb = const.tile([128, NT], I32)
            nc.sync.dma_start(out=idx_sb[:], in_=idx_d[:])
            wi0 = const.tile([128, ET, H], BF16)
            nc.sync.dma_start(out=wi0[:], in_=wi0_d[:])
            wh0 = const.tile([128, KT, H], BF16)
            nc.sync.dma_start(out=wh0[:], in_=wh0_d[:])
            wi1 = const.tile([128, KT, H], BF16)
            nc.sync.dma_start(out=wi1[:], in_=wi1_d[:])
            wh1 = const.tile([128, KT, H], BF16)
            nc.sync.dma_start(out=wh1[:], in_=wh1_d[:])
            bias0 = const.tile([1, H], BF16)
            nc.sync.dma_start(out=bias0[:], in_=b0_d[:])
            bias1 = const.tile([1, H], BF16)
            nc.sync.dma_start(out=bias1[:], in_=b1_d[:])
            w12 = const.tile([128, KT, C], F32)
            nc.sync.dma_start(out=w12[:], in_=w12_d[:])

            id128 = const.tile([128, 128], F32)
            make_identity(nc, id128[:])
            id32 = const.tile([BL, BL], BF16)
            make_identity(nc, id32[:])
            ones = const.tile([1, 128], BF16)
            nc.vector.memset(ones[:], 1.0)
            szero = const.tile([128, KT, BL], BF16)
            nc.vector.memset(szero[:], 0.0)

            def xpart(dst, lhs_tiles, w, bias, nk):
                """dst[:, i, :] (bf16 [128, GPC, H]) = x-part for tb-tile i."""
                for i in range(GPC):
                    for h in range(2):
                        ps = ppsp.tile([128, 512], F32, tag="pps")
                        for k in range(nk):
                            nc.tensor.matmul(
                                out=ps[:],
                                lhsT=lhs_tiles(k, i),
                                rhs=w[:, k, 512 * h:512 * h + 512],
                                start=(k == 0), stop=False,
                            )
                        nc.tensor.matmul(
                            out=ps[:], lhsT=ones[:],
                            rhs=bias[:, 512 * h:512 * h + 512],
                            start=False, stop=True,
                        )
                        nc.vector.tensor_copy(
                            out=dst[:, i, 512 * h:512 * h + 512], in_=ps[:])

            def scan_step(src_tile, src_col, p, t, wh, store):
                """One recurrent step: store(tanh(src^T.T @ Wh + p_t))."""
                sps = spsp.tile([BL, H], F32, tag="sps")
                for h in range(2):
                    for k in range(KT):
                        nc.tensor.matmul(
                            out=sps[:, 512 * h:512 * h + 512],
                            lhsT=src_tile[:, k, src_col:src_col + BL],
                            rhs=wh[:, k, 512 * h:512 * h + 512],
                            start=(k == 0), stop=(k == KT - 1),
                        )
                a = actp.tile([BL, H], BF16, tag="act")
                pt = p[32 * (t % 4):32 * (t % 4) + 32, t // 4, :]
                for h in range(2):
                    sl = slice(512 * h, 512 * h + 512)
                    nc.vector.tensor_tensor(
                        out=sps[:, sl], in0=sps[:, sl], in1=pt[:, sl],
                        op=mybir.AluOpType.add)
                    nc.scalar.activation(
                        out=a[:, sl], in_=sps[:, sl],
                        func=mybir.ActivationFunctionType.Tanh)
                trs = trsp.tile([128, KT, BL], BF16, tag="trs")
                for j in range(KT):
                    nc.tensor.matmul(
                        out=trs[:, j, :], lhsT=a[:, 128 * j:128 * j + 128],
                        rhs=id32[:], is_transpose=True, start=True, stop=True)
                store(trs)

            st0_tile, st0_col = szero, 0   # layer-0 state source (tile, col)
            st1_tile, st1_col = szero, 0
            for c in range(NCH):
                # ---- gather + transpose e^T for this chunk ----
                eT = etp.tile([128, ET, T * BL], BF16, tag="et")
                for i in range(GPC):
                    g = gat.tile([128, E], F32, tag="gat")
                    nc.gpsimd.indirect_dma_start(
                        out=g[:], out_offset=None,
                        in_=emb_d[:],
                        in_offset=bass.IndirectOffsetOnAxis(
                            ap=idx_sb[:, c * GPC + i:c * GPC + i + 1], axis=0),
                    )
                    for j in range(ET):
                        etr = trsp.tile([128, 128], F32, tag="trs")
                        nc.tensor.matmul(
                            out=etr[:], lhsT=g[:, 128 * j:128 * j + 128],
                            rhs=id128[:], is_transpose=True,
                            start=True, stop=True)
                        nc.vector.tensor_copy(
                            out=eT[:, j, 128 * i:128 * i + 128], in_=etr[:])
                # ---- P0 ----
                p0 = p0p.tile([128, GPC, H], BF16, tag="p0")
                xpart(p0, lambda k, i: eT[:, k, 128 * i:128 * i + 128],
                      wi0, bias0, ET)
                # ---- layer-0 scan ----
                ch0 = ch0p.tile([128, KT, T * BL], BF16, tag="ch0")
                for tl in range(T):
                    def store0(trs, tl=tl, ch0=ch0):
                        nc.vector.tensor_copy(
                            out=ch0[:, :, BL * tl:BL * tl + BL], in_=trs[:])
                    scan_step(st0_tile, st0_col, p0, tl, wh0, store0)
                    st0_tile, st0_col = ch0, BL * tl
                # ---- P1 ----
                p1 = p1p.tile([128, GPC, H], BF16, tag="p1")
                xpart(p1, lambda k, i: ch0[:, k, 128 * i:128 * i + 128],
                      wi1, bias1, KT)
                # ---- layer-1 scan ----
                for tl in range(T):
                    st_new = st1p.tile([128, KT, BL], BF16, tag="st1")

                    def store1(trs, st_new=st_new):
                        nc.vector.tensor_copy(out=st_new[:], in_=trs[:])

                    scan_step(st1_tile, st1_col, p1, tl, wh1, store1)
                    st1_tile, st1_col = st_new, 0

            # ---- head: out = h1_final^T.T @ w12_half ----
            hf = actp.tile([128, KT, BL], F32, tag="hf")
            nc.vector.tensor_copy(
                out=hf[:], in_=st1_tile[:, :, st1_col:st1_col + BL])
            hps = ppsp.tile([BL, C], F32, tag="pps")
            for k in range(KT):
                nc.tensor.matmul(out=hps[:], lhsT=hf[:, k, :], rhs=w12[:, k, :],
                                 start=(k == 0), stop=(k == KT - 1))
            outsb = osb.tile([BL, C], F32)
            nc.vector.tensor_copy(out=outsb[:], in_=hps[:])
            nc.sync.dma_start(out=out_d[:], in_=outsb[:])

    nc.compile()
    return nc


def _pack_dir(inputs, d, S_):
    """Per-direction constant tensors (shared by that direction's 4 cores)."""
    bf = ml_dtypes.bfloat16
    p = ("f" if d == 0 else "b")
    wi0 = np.ascontiguousarray(
        np.asarray(inputs[p + "0_Wi"], np.float32).reshape(ET, 128, H)
        .transpose(1, 0, 2)).astype(bf)
    wh0 = np.ascontiguousarray(
        np.asarray(inputs[p + "0_Wh"], np.float32).reshape(KT, 128, H)
        .transpose(1, 0, 2)).astype(bf)
    wi1 = np.ascontiguousarray(
        np.asarray(inputs[p + "1_Wi"], np.float32).reshape(KT, 128, H)
        .transpose(1, 0, 2)).astype(bf)
    wh1 = np.ascontiguousarray(
        np.asarray(inputs[p + "1_Wh"], np.float32).reshape(KT, 128, H)
        .transpose(1, 0, 2)).astype(bf)
    b0 = (np.asarray(inputs[p + "0_bi"], np.float32)
          + np.asarray(inputs[p + "0_bh"], np.float32)).reshape(1, H).astype(bf)
    b1 = (np.asarray(inputs[p + "1_bi"], np.float32)
          + np.asarray(inputs[p + "1_bh"], np.float32)).reshape(1, H).astype(bf)
    W12 = (np.asarray(inputs["fc1_W"], np.float64)
           @ np.asarray(inputs["fc2_W"], np.float64))
    w12h = np.ascontiguousarray(
        W12[d * H:(d + 1) * H].reshape(KT, 128, C).transpose(1, 0, 2)
    ).astype(np.float32)
    return dict(wi0=wi0, wh0=wh0, wi1=wi1, wh1=wh1, bias0=b0, bias1=b1,
                w12=w12h)


def _make_idx(x, q, d, S_, NT):
    """[128, NT] int32 gather indices for core (direction d, quarter q)."""
    p = np.arange(128)
    tau = np.arange(NT)
    t = (p[:, None] // BL) + 4 * tau[None, :]
    if d == 1:
        t = (S_ - 1) - t
    b = q * BL + (p[:, None] % BL) + 0 * tau[None, :]
    return np.ascontiguousarray(x[b, t].astype(np.int32))


def kernel(**inputs):
    S_ = int(np.asarray(inputs["x"]).shape[1])
    key = ("nc", S_)
    if key not in _CACHE:
        _CACHE[key] = _build(S_=S_)
    nc = _CACHE[key]
    NT = (S_ * BL) // 128

    x = np.asarray(inputs["x"]).astype(np.int64)
    emb = np.ascontiguousarray(np.asarray(inputs["emb"], np.float32))
    packs = [_pack_dir(inputs, d, S_) for d in (0, 1)]

    in_maps = []
    for core in range(8):
        d, q = divmod(core, 4)
        m = dict(packs[d])
        m["emb"] = emb
        m["idx"] = _make_idx(x, q, d, S_, NT)
        in_maps.append(m)

    res = run_bass_kernel_spmd(nc, in_maps, list(range(8))).results

    W12b = (np.asarray(inputs["fc1_b"], np.float64)
            @ np.asarray(inputs["fc2_W"], np.float64)
            + np.asarray(inputs["fc2_b"], np.float64)).astype(np.float32)
    out = np.empty((B, C), np.float32)
    for q in range(4):
        out[q * BL:(q + 1) * BL] = (res[q]["out"] + res[4 + q]["out"] + W12b)
    return out


# revision 11
# speedup vs baseline: 1.5826x; 1.5826x over previous
"""Trainium2 Bass kernel for a 2-layer bidirectional RNN (tanh cells) + linear head.

Model (hardcoded shapes): B=128, S=512, E=512, H=1024, V=50000, C=2.

Sharding: 8 cores = 2 directions x 4 batch-quarters (B_local=32). Each core runs
both stacked layers of its direction over the full sequence, chunk by chunk:
  gather emb rows -> PE-transpose to e^T -> P0 = e @ Wi0 + biases (big matmul)
  -> layer-0 scan (h^T kept transposed; Wh streamed at N=512)
  -> P1 = h0 @ Wi1 + biases -> layer-1 scan -> final state -> head partial.
The two fc layers have no nonlinearity between them, so they collapse to a
single [2H, 2] matrix W12 = fc1_W @ fc2_W; each core computes its direction's
partial product on device and the host sums partials + b12 during unshard.
All matmuls run in bf16 with fp32 PSUM accumulation (validated: rel err ~3e-3).
"""

import numpy as np
import ml_dtypes

import concourse.bass as bass
import concourse.mybir as mybir
import concourse.tile as tile
from concourse import bacc
from concourse.masks import make_identity
from concourse.bass_utils import run_bass_kernel_spmd

F32 = mybir.dt.float32
BF16 = mybir.dt.bfloat16
I32 = mybir.dt.int32

B, S, E, H, V, C = 128, 512, 512, 1024, 50000, 2
BL = 32          # batch per core
KT = H // 128    # 8 k-tiles of the hidden dim
ET = E // 128    # 4 k-tiles of the embedding dim

_CACHE = {}


def _build(S_=S, T=16):
    """Build the (uniform, per-core) Bass program. T = steps per chunk."""
    NCH = S_ // T            # chunks
    GPC = (T * BL) // 128    # gather tiles per chunk (tb-tiles)
    NT = NCH * GPC           # total tb-tiles

    nc = bacc.Bacc(None, target_bir_lowering=False, debug=True)

    emb_d = nc.dram_tensor("emb", [V, E], BF16, kind="ExternalInput")
    idx_d = nc.dram_tensor("idx", [128, NT], I32, kind="ExternalInput")
    wi0_d = nc.dram_tensor("wi0", [128, ET, H], BF16, kind="ExternalInput")
    wh0_d = nc.dram_tensor("wh0", [128, KT, H], BF16, kind="ExternalInput")
    wi1_d = nc.dram_tensor("wi1", [128, KT, H], BF16, kind="ExternalInput")
    wh1_d = nc.dram_tensor("wh1", [128, KT, H], BF16, kind="ExternalInput")
    b0_d = nc.dram_tensor("bias0", [1, H], BF16, kind="ExternalInput")
    b1_d = nc.dram_tensor("bias1", [1, H], BF16, kind="ExternalInput")
    w12_d = nc.dram_tensor("w12", [128, KT, C], F32, kind="ExternalInput")
    out_d = nc.dram_tensor("out", [BL, C], F32, kind="ExternalOutput")

    with tile.TileContext(nc) as tc:
        with (
            tc.tile_pool(name="const", bufs=1) as const,
            tc.tile_pool(name="gat", bufs=4) as gat,
            tc.tile_pool(name="et", bufs=2) as etp,
            tc.tile_pool(name="p0", bufs=2) as p0p,
            tc.tile_pool(name="p1", bufs=2) as p1p,
            tc.tile_pool(name="ch0", bufs=2) as ch0p,
            tc.tile_pool(name="st1", bufs=2) as st1p,
            tc.tile_pool(name="act", bufs=2) as actp,
            tc.tile_pool(name="osb", bufs=1) as osb,
            tc.tile_pool(name="sps", bufs=2, space="PSUM") as spsp,
            tc.tile_pool(name="trs", bufs=2, space="PSUM") as trsp,
            tc.tile_pool(name="pps", bufs=2, space="PSUM") as ppsp,
        ):
            # ---- constants / weights into SBUF ----
            idx_sb = const.tile([128, NT], I32)
            nc.sync.dma_start(out=idx_sb[:], in_=idx_d[:])
            wi0 = const.tile([128, ET, H], BF16)
            nc.sync.dma_start(out=wi0[:], in_=wi0_d[:])
            wh0 = const.tile([128, KT, H], BF16)
            nc.sync.dma_start(out=wh0[:], in_=wh0_d[:])
            wi1 = const.tile([128, KT, H], BF16)
            nc.sync.dma_start(out=wi1[:], in_=wi1_d[:])
            wh1 = const.tile([128, KT, H], BF16)
            nc.sync.dma_start(out=wh1[:], in_=wh1_d[:])
            bias0 = const.tile([1, H], BF16)
            nc.sync.dma_start(out=bias0[:], in_=b0_d[:])
            bias1 = const.tile([1, H], BF16)
            nc.sync.dma_start(out=bias1[:], in_=b1_d[:])
            w12 = const.tile([128, KT, C], F32)
            nc.sync.dma_start(out=w12[:], in_=w12_d[:])

            id128 = const.tile([128, 128], BF16)
            make_identity(nc, id128[:])
            id32 = const.tile([BL, BL], BF16)
            make_identity(nc, id32[:])
            ones = const.tile([1, 128], BF16)
            nc.vector.memset(ones[:], 1.0)
            szero = const.tile([128, KT, BL], BF16)
            nc.vector.memset(szero[:], 0.0)

            def xpart(dst, lhs_tiles, w, bias, nk):
                """dst[:, i, :] (bf16 [128, GPC, H]) = x-part for tb-tile i."""
                for i in range(GPC):
                    for h in range(2):
                        ps = ppsp.tile([128, 512], F32, tag="pps")
                        for k in range(nk):
                            nc.tensor.matmul(
                                out=ps[:],
                                lhsT=lhs_tiles(k, i),
                                rhs=w[:, k, 512 * h:512 * h + 512],
                                start=(k == 0), stop=False,
                            )
                        nc.tensor.matmul(
                            out=ps[:], lhsT=ones[:],
                            rhs=bias[:, 512 * h:512 * h + 512],
                            start=False, stop=True,
                        )
                        nc.vector.tensor_copy(
                            out=dst[:, i, 512 * h:512 * h + 512], in_=ps[:])

            def scan_step(src_tile, src_col, p, t, wh, store):
                """One recurrent step: store(tanh(src^T.T @ Wh + p_t))."""
                sps = spsp.tile([BL, H], F32, tag="sps")
                for h in range(2):
                    for k in range(KT):
                        nc.tensor.matmul(
                            out=sps[:, 512 * h:512 * h + 512],
                            lhsT=src_tile[:, k, src_col:src_col + BL],
                            rhs=wh[:, k, 512 * h:512 * h + 512],
                            start=(k == 0), stop=(k == KT - 1),
                        )
                a = actp.tile([BL, H], BF16, tag="act")
                pt = p[32 * (t % 4):32 * (t % 4) + 32, t // 4, :]
                for h in range(2):
                    sl = slice(512 * h, 512 * h + 512)
                    nc.vector.tensor_tensor(
                        out=sps[:, sl], in0=sps[:, sl], in1=pt[:, sl],
                        op=mybir.AluOpType.add)
                    nc.scalar.activation(
                        out=a[:, sl], in_=sps[:, sl],
                        func=mybir.ActivationFunctionType.Tanh)
                trs = trsp.tile([128, KT, BL], BF16, tag="trs")
                for j in range(KT):
                    nc.tensor.matmul(
                        out=trs[:, j, :], lhsT=a[:, 128 * j:128 * j + 128],
                        rhs=id32[:], is_transpose=True, start=True, stop=True)
                store(trs)

            st0_tile, st0_col = szero, 0   # layer-0 state source (tile, col)
            st1_tile, st1_col = szero, 0
            p1_prev = None   # P1 of chunk c-1 (layer 1 runs one chunk behind)

            def l1_step(p1c, tl):
                nonlocal st1_tile, st1_col
                st_new = st1p.tile([128, KT, BL], BF16, tag="st1")

                def store1(trs, st_new=st_new):
                    nc.vector.tensor_copy(out=st_new[:], in_=trs[:])

                scan_step(st1_tile, st1_col, p1c, tl, wh1, store1)
                st1_tile, st1_col = st_new, 0

            for c in range(NCH):
                # ---- gather + transpose e^T for this chunk ----
                eT = etp.tile([128, ET, T * BL], BF16, tag="et")
                for i in range(GPC):
                    g = gat.tile([128, E], BF16, tag="gat")
                    nc.gpsimd.indirect_dma_start(
                        out=g[:], out_offset=None,
                        in_=emb_d[:],
                        in_offset=bass.IndirectOffsetOnAxis(
                            ap=idx_sb[:, c * GPC + i:c * GPC + i + 1], axis=0),
                    )
                    for j in range(ET):
                        etr = trsp.tile([128, 128], BF16, tag="trs")
                        nc.tensor.matmul(
                            out=etr[:], lhsT=g[:, 128 * j:128 * j + 128],
                            rhs=id128[:], is_transpose=True,
                            start=True, stop=True)
                        nc.vector.tensor_copy(
                            out=eT[:, j, 128 * i:128 * i + 128], in_=etr[:])
                # ---- P0 ----
                p0 = p0p.tile([128, GPC, H], BF16, tag="p0")
                xpart(p0, lambda k, i: eT[:, k, 128 * i:128 * i + 128],
                      wi0, bias0, ET)
                # ---- interleaved scans: L0 chunk c, L1 chunk c-1 ----
                ch0 = ch0p.tile([128, KT, T * BL], BF16, tag="ch0")
                for tl in range(T):
                    def store0(trs, tl=tl, ch0=ch0):
                        nc.vector.tensor_copy(
                            out=ch0[:, :, BL * tl:BL * tl + BL], in_=trs[:])
                    scan_step(st0_tile, st0_col, p0, tl, wh0, store0)
                    st0_tile, st0_col = ch0, BL * tl
                    if p1_prev is not None:
                        l1_step(p1_prev, tl)
                # ---- P1 of chunk c (consumed next iteration) ----
                p1 = p1p.tile([128, GPC, H], BF16, tag="p1")
                xpart(p1, lambda k, i: ch0[:, k, 128 * i:128 * i + 128],
                      wi1, bias1, KT)
                p1_prev = p1
            # ---- drain: layer-1 over the last chunk ----
            for tl in range(T):
                l1_step(p1_prev, tl)

            # ---- head: out = h1_final^T.T @ w12_half ----
            hf = actp.tile([128, KT, BL], F32, tag="hf")
            nc.vector.tensor_copy(
                out=hf[:], in_=st1_tile[:, :, st1_col:st1_col + BL])
            hps = ppsp.tile([BL, C], F32, tag="pps")
            for k in range(KT):
                nc.tensor.matmul(out=hps[:], lhsT=hf[:, k, :], rhs=w12[:, k, :],
                                 start=(k == 0), stop=(k == KT - 1))
            outsb = osb.tile([BL, C], F32)
            nc.vector.tensor_copy(out=outsb[:], in_=hps[:])
            nc.sync.dma_start(out=out_d[:], in_=outsb[:])

    nc.compile()
    return nc


def _pack_dir(inputs, d, S_):
    """Per-direction constant tensors (shared by that direction's 4 cores)."""
    bf = ml_dtypes.bfloat16
    p = ("f" if d == 0 else "b")
    wi0 = np.ascontiguousarray(
        np.asarray(inputs[p + "0_Wi"], np.float32).reshape(ET, 128, H)
        .transpose(1, 0, 2)).astype(bf)
    wh0 = np.ascontiguousarray(
        np.asarray(inputs[p + "0_Wh"], np.float32).reshape(KT, 128, H)
        .transpose(1, 0, 2)).astype(bf)
    wi1 = np.ascontiguousarray(
        np.asarray(inputs[p + "1_Wi"], np.float32).reshape(KT, 128, H)
        .transpose(1, 0, 2)).astype(bf)
    wh1 = np.ascontiguousarray(
        np.asarray(inputs[p + "1_Wh"], np.float32).reshape(KT, 128, H)
        .transpose(1, 0, 2)).astype(bf)
    b0 = (np.asarray(inputs[p + "0_bi"], np.float32)
          + np.asarray(inputs[p + "0_bh"], np.float32)).reshape(1, H).astype(bf)
    b1 = (np.asarray(inputs[p + "1_bi"], np.float32)
          + np.asarray(inputs[p + "1_bh"], np.float32)).reshape(1, H).astype(bf)
    W12 = (np.asarray(inputs["fc1_W"], np.float64)
           @ np.asarray(inputs["fc2_W"], np.float64))
    w12h = np.ascontiguousarray(
        W12[d * H:(d + 1) * H].reshape(KT, 128, C).transpose(1, 0, 2)
    ).astype(np.float32)
    return dict(wi0=wi0, wh0=wh0, wi1=wi1, wh1=wh1, bias0=b0, bias1=b1,
                w12=w12h)


def _make_idx(x, q, d, S_, NT):
    """[128, NT] int32 gather indices for core (direction d, quarter q)."""
    p = np.arange(128)
    tau = np.arange(NT)
    t = (p[:, None] // BL) + 4 * tau[None, :]
    if d == 1:
        t = (S_ - 1) - t
    b = q * BL + (p[:, None] % BL) + 0 * tau[None, :]
    return np.ascontiguousarray(x[b, t].astype(np.int32))


def kernel(**inputs):
    S_ = int(np.asarray(inputs["x"]).shape[1])
    key = ("nc", S_)
    if key not in _CACHE:
        _CACHE[key] = _build(S_=S_)
    nc = _CACHE[key]
    NT = (S_ * BL) // 128

    x = np.asarray(inputs["x"]).astype(np.int64)
    emb_bf = np.ascontiguousarray(np.asarray(inputs["emb"], np.float32).astype(ml_dtypes.bfloat16))
    packs = [_pack_dir(inputs, d, S_) for d in (0, 1)]

    in_maps = []
    for core in range(8):
        d, q = divmod(core, 4)
        m = dict(packs[d])
        m["emb"] = emb_bf
        m["idx"] = _make_idx(x, q, d, S_, NT)
        in_maps.append(m)

    import os
    kw = {}
    if os.environ.get("BIRNN_TRACE"):
        kw = dict(trace=True, tmpdir=os.environ.get("BIRNN_TRACE"))
    r = run_bass_kernel_spmd(nc, in_maps, list(range(8)), **kw)
    globals()["LAST_EXEC_NS"] = getattr(r, "exec_time_ns", None)
    globals()["LAST_RESULT"] = r
    res = r.results

    W12b = (np.asarray(inputs["fc1_b"], np.float64)
            @ np.asarray(inputs["fc2_W"], np.float64)
            + np.asarray(inputs["fc2_b"], np.float64)).astype(np.float32)
    out = np.empty((B, C), np.float32)
    for q in range(4):
        out[q * BL:(q + 1) * BL] = (res[q]["out"] + res[4 + q]["out"] + W12b)
    return out


# revision 14
# speedup vs baseline: 3836.8170x; 2424.3401x over previous
"""Trainium2 Bass kernel for a 2-layer bidirectional RNN (tanh cells) + linear head.

Model (hardcoded shapes): B=128, S=512, E=512, H=1024, V=50000, C=2.

Sharding: 8 cores = 2 directions x 4 batch-quarters (B_local=32). Each core runs
both stacked layers of its direction over the full sequence, chunk by chunk:
  gather emb rows -> PE-transpose to e^T -> P0 = e @ Wi0 + biases (big matmul)
  -> layer-0 scan (h^T kept transposed; Wh streamed at N=512)
  -> P1 = h0 @ Wi1 + biases -> layer-1 scan -> final state -> head partial.
The two fc layers have no nonlinearity between them, so they collapse to a
single [2H, 2] matrix W12 = fc1_W @ fc2_W; each core computes its direction's
partial product on device and the host sums partials + b12 during unshard.
All matmuls run in bf16 with fp32 PSUM accumulation (validated: rel err ~3e-3).
"""

import numpy as np
import ml_dtypes

import concourse.bass as bass
import concourse.mybir as mybir
import concourse.tile as tile
from concourse import bacc
from concourse.masks import make_identity
from concourse.bass_utils import run_bass_kernel_spmd

F32 = mybir.dt.float32
BF16 = mybir.dt.bfloat16
I32 = mybir.dt.int32

B, S, E, H, V, C = 128, 512, 512, 1024, 50000, 2
BL = 32          # batch per core
KT = H // 128    # 8 k-tiles of the hidden dim
ET = E // 128    # 4 k-tiles of the embedding dim

_CACHE = {}


def _build(S_=S, T=16):
    """Build the (uniform, per-core) Bass program. T = steps per chunk."""
    NCH = S_ // T            # chunks
    GPC = (T * BL) // 128    # gather tiles per chunk (tb-tiles)
    NT = NCH * GPC           # total tb-tiles

    nc = bacc.Bacc(None, target_bir_lowering=False, debug=True)

    emb_d = nc.dram_tensor("emb", [V, E], BF16, kind="ExternalInput")
    idx_d = nc.dram_tensor("idx", [128, NT], I32, kind="ExternalInput")
    wi0_d = nc.dram_tensor("wi0", [128, ET, H], BF16, kind="ExternalInput")
    wh0_d = nc.dram_tensor("wh0", [128, KT, H], BF16, kind="ExternalInput")
    wi1_d = nc.dram_tensor("wi1", [128, KT, H], BF16, kind="ExternalInput")
    wh1_d = nc.dram_tensor("wh1", [128, KT, H], BF16, kind="ExternalInput")
    b0_d = nc.dram_tensor("bias0", [1, H], BF16, kind="ExternalInput")
    b1_d = nc.dram_tensor("bias1", [1, H], BF16, kind="ExternalInput")
    w12_d = nc.dram_tensor("w12", [128, KT, C], F32, kind="ExternalInput")
    out_d = nc.dram_tensor("out", [BL, C], F32, kind="ExternalOutput")

    with tile.TileContext(nc) as tc:
        with (
            tc.tile_pool(name="const", bufs=1) as const,
            tc.tile_pool(name="gat", bufs=4) as gat,
            tc.tile_pool(name="et", bufs=2) as etp,
            tc.tile_pool(name="p0", bufs=2) as p0p,
            tc.tile_pool(name="p1", bufs=2) as p1p,
            tc.tile_pool(name="ch0", bufs=2) as ch0p,
            tc.tile_pool(name="st1", bufs=2) as st1p,
            tc.tile_pool(name="act", bufs=2) as actp,
            tc.tile_pool(name="osb", bufs=1) as osb,
            tc.tile_pool(name="sps", bufs=2, space="PSUM") as spsp,
            tc.tile_pool(name="trs", bufs=2, space="PSUM") as trsp,
            tc.tile_pool(name="pps", bufs=2, space="PSUM") as ppsp,
        ):
            # ---- constants / weights into SBUF ----
            idx_sb = const.tile([128, NT], I32)
            nc.sync.dma_start(out=idx_sb[:], in_=idx_d[:])
            wi0 = const.tile([128, ET, H], BF16)
            nc.sync.dma_start(out=wi0[:], in_=wi0_d[:])
            wh0 = const.tile([128, KT, H], BF16)
            nc.sync.dma_start(out=wh0[:], in_=wh0_d[:])
            wi1 = const.tile([128, KT, H], BF16)
            nc.sync.dma_start(out=wi1[:], in_=wi1_d[:])
            wh1 = const.tile([128, KT, H], BF16)
            nc.sync.dma_start(out=wh1[:], in_=wh1_d[:])
            bias0 = const.tile([1, H], BF16)
            nc.sync.dma_start(out=bias0[:], in_=b0_d[:])
            bias1 = const.tile([1, H], BF16)
            nc.sync.dma_start(out=bias1[:], in_=b1_d[:])
            w12 = const.tile([128, KT, C], F32)
            nc.sync.dma_start(out=w12[:], in_=w12_d[:])

            id128 = const.tile([128, 128], BF16)
            make_identity(nc, id128[:])
            id32 = const.tile([BL, BL], BF16)
            make_identity(nc, id32[:])
            ones = const.tile([1, 128], BF16)
            nc.vector.memset(ones[:], 1.0)
            szero = const.tile([128, KT, BL], BF16)
            nc.vector.memset(szero[:], 0.0)

            def xpart(dst, lhs_tiles, w, bias, nk):
                """dst[:, i, :] (bf16 [128, GPC, H]) = x-part for tb-tile i."""
                for i in range(GPC):
                    for h in range(2):
                        ps = ppsp.tile([128, 512], F32, tag="pps")
                        for k in range(nk):
                            nc.tensor.matmul(
                                out=ps[:],
                                lhsT=lhs_tiles(k, i),
                                rhs=w[:, k, 512 * h:512 * h + 512],
                                start=(k == 0), stop=False,
                            )
                        nc.tensor.matmul(
                            out=ps[:], lhsT=ones[:],
                            rhs=bias[:, 512 * h:512 * h + 512],
                            start=False, stop=True,
                        )
                        nc.vector.tensor_copy(
                            out=dst[:, i, 512 * h:512 * h + 512], in_=ps[:])

            def scan_step(src_tile, src_col, p, t, wh, store):
                """One recurrent step: store(tanh(src^T.T @ Wh + p_t))."""
                sps = spsp.tile([BL, H], F32, tag="sps")
                for h in range(2):
                    for k in range(KT):
                        nc.tensor.matmul(
                            out=sps[:, 512 * h:512 * h + 512],
                            lhsT=src_tile[:, k, src_col:src_col + BL],
                            rhs=wh[:, k, 512 * h:512 * h + 512],
                            start=(k == 0), stop=(k == KT - 1),
                        )
                a = actp.tile([BL, H], BF16, tag="act")
                pt = p[32 * (t % 4):32 * (t % 4) + 32, t // 4, :]
                for h in range(2):
                    sl = slice(512 * h, 512 * h + 512)
                    nc.vector.tensor_tensor(
                        out=sps[:, sl], in0=sps[:, sl], in1=pt[:, sl],
                        op=mybir.AluOpType.add)
                    nc.scalar.activation(
                        out=a[:, sl], in_=sps[:, sl],
                        func=mybir.ActivationFunctionType.Tanh)
                trs = trsp.tile([128, KT, BL], BF16, tag="trs")
                for j in range(KT):
                    nc.tensor.matmul(
                        out=trs[:, j, :], lhsT=a[:, 128 * j:128 * j + 128],
                        rhs=id32[:], is_transpose=True, start=True, stop=True)
                store(trs)

            st0_tile, st0_col = szero, 0   # layer-0 state source (tile, col)
            st1_tile, st1_col = szero, 0
            p1_prev = None   # P1 of chunk c-1 (layer 1 runs one chunk behind)

            def l1_step(p1c, tl):
                nonlocal st1_tile, st1_col
                st_new = st1p.tile([128, KT, BL], BF16, tag="st1")

                def store1(trs, st_new=st_new):
                    nc.vector.tensor_copy(out=st_new[:], in_=trs[:])

                scan_step(st1_tile, st1_col, p1c, tl, wh1, store1)
                st1_tile, st1_col = st_new, 0

            for c in range(NCH):
                # ---- gather + transpose e^T for this chunk ----
                eT = etp.tile([128, ET, T * BL], BF16, tag="et")
                for i in range(GPC):
                    g = gat.tile([128, E], BF16, tag="gat")
                    nc.gpsimd.indirect_dma_start(
                        out=g[:], out_offset=None,
                        in_=emb_d[:],
                        in_offset=bass.IndirectOffsetOnAxis(
                            ap=idx_sb[:, c * GPC + i:c * GPC + i + 1], axis=0),
                    )
                    for j in range(ET):
                        etr = trsp.tile([128, 128], BF16, tag="trs")
                        nc.tensor.matmul(
                            out=etr[:], lhsT=g[:, 128 * j:128 * j + 128],
                            rhs=id128[:], is_transpose=True,
                            start=True, stop=True)
                        nc.vector.tensor_copy(
                            out=eT[:, j, 128 * i:128 * i + 128], in_=etr[:])
                # ---- P0 ----
                p0 = p0p.tile([128, GPC, H], BF16, tag="p0")
                xpart(p0, lambda k, i: eT[:, k, 128 * i:128 * i + 128],
                      wi0, bias0, ET)
                # ---- interleaved scans: L0 chunk c, L1 chunk c-1 ----
                ch0 = ch0p.tile([128, KT, T * BL], BF16, tag="ch0")
                for tl in range(T):
                    def store0(trs, tl=tl, ch0=ch0):
                        nc.vector.tensor_copy(
                            out=ch0[:, :, BL * tl:BL * tl + BL], in_=trs[:])
                    scan_step(st0_tile, st0_col, p0, tl, wh0, store0)
                    st0_tile, st0_col = ch0, BL * tl
                    if p1_prev is not None:
                        l1_step(p1_prev, tl)
                # ---- P1 of chunk c (consumed next iteration) ----
                p1 = p1p.tile([128, GPC, H], BF16, tag="p1")
                xpart(p1, lambda k, i: ch0[:, k, 128 * i:128 * i + 128],
                      wi1, bias1, KT)
                p1_prev = p1
            # ---- drain: layer-1 over the last chunk ----
            for tl in range(T):
                l1_step(p1_prev, tl)

            # ---- head: out = h1_final^T.T @ w12_half ----
            hf = actp.tile([128, KT, BL], F32, tag="hf")
            nc.vector.tensor_copy(
                out=hf[:], in_=st1_tile[:, :, st1_col:st1_col + BL])
            hps = ppsp.tile([BL, C], F32, tag="pps")
            for k in range(KT):
                nc.tensor.matmul(out=hps[:], lhsT=hf[:, k, :], rhs=w12[:, k, :],
                                 start=(k == 0), stop=(k == KT - 1))
            outsb = osb.tile([BL, C], F32)
            nc.vector.tensor_copy(out=outsb[:], in_=hps[:])
            nc.sync.dma_start(out=out_d[:], in_=outsb[:])

    nc.compile()
    return nc


def _pack_dir(inputs, d, S_):
    """Per-direction constant tensors (shared by that direction's 4 cores)."""
    bf = ml_dtypes.bfloat16
    p = ("f" if d == 0 else "b")
    wi0 = np.ascontiguousarray(
        np.asarray(inputs[p + "0_Wi"], np.float32).reshape(ET, 128, H)
        .transpose(1, 0, 2)).astype(bf)
    wh0 = np.ascontiguousarray(
        np.asarray(inputs[p + "0_Wh"], np.float32).reshape(KT, 128, H)
        .transpose(1, 0, 2)).astype(bf)
    wi1 = np.ascontiguousarray(
        np.asarray(inputs[p + "1_Wi"], np.float32).reshape(KT, 128, H)
        .transpose(1, 0, 2)).astype(bf)
    wh1 = np.ascontiguousarray(
        np.asarray(inputs[p + "1_Wh"], np.float32).reshape(KT, 128, H)
        .transpose(1, 0, 2)).astype(bf)
    b0 = (np.asarray(inputs[p + "0_bi"], np.float32)
          + np.asarray(inputs[p + "0_bh"], np.float32)).reshape(1, H).astype(bf)
    b1 = (np.asarray(inputs[p + "1_bi"], np.float32)
          + np.asarray(inputs[p + "1_bh"], np.float32)).reshape(1, H).astype(bf)
    W12 = (np.asarray(inputs["fc1_W"], np.float64)
           @ np.asarray(inputs["fc2_W"], np.float64))
    w12h = np.ascontiguousarray(
        W12[d * H:(d + 1) * H].reshape(KT, 128, C).transpose(1, 0, 2)
    ).astype(np.float32)
    return dict(wi0=wi0, wh0=wh0, wi1=wi1, wh1=wh1, bias0=b0, bias1=b1,
                w12=w12h)


def _make_idx(x, q, d, S_, NT):
    """[128, NT] int32 gather indices for core (direction d, quarter q)."""
    p = np.arange(128)
    tau = np.arange(NT)
    t = (p[:, None] // BL) + 4 * tau[None, :]
    if d == 1:
        t = (S_ - 1) - t
    b = q * BL + (p[:, None] % BL) + 0 * tau[None, :]
    return np.ascontiguousarray(x[b, t].astype(np.int32))


def kernel(**inputs):
    S_ = int(np.asarray(inputs["x"]).shape[1])
    key = ("nc", S_)
    if key not in _CACHE:
        _CACHE[key] = _build(S_=S_)
    nc = _CACHE[key]
    NT = (S_ * BL) // 128

    x = np.asarray(inputs["x"]).astype(np.int64)
    emb_bf = np.ascontiguousarray(np.asarray(inputs["emb"], np.float32).astype(ml_dtypes.bfloat16))
    packs = [_pack_dir(inputs, d, S_) for d in (0, 1)]

    in_maps = []
    for core in range(8):
        d, q = divmod(core, 4)
        m = dict(packs[d])
        m["emb"] = emb_bf
        m["idx"] = _make_idx(x, q, d, S_, NT)
        in_maps.append(m)

    import os
    kw = {}
    if os.environ.get("BIRNN_TRACE"):
        kw = dict(trace=True, tmpdir=os.environ.get("BIRNN_TRACE"))
    r = run_bass_kernel_spmd(nc, in_maps, list(range(8)), **kw)
    globals()["LAST_EXEC_NS"] = getattr(r, "exec_time_ns", None)
    globals()["LAST_RESULT"] = r
    res = r.results

    W12b = (np.asarray(inputs["fc1_b"], np.float64)
            @ np.asarray(inputs["fc2_W"], np.float64)
            + np.asarray(inputs["fc2_b"], np.float64)).astype(np.float32)
    out = np.empty((B, C), np.float32)
    for q in range(4):
        out[q * BL:(q + 1) * BL] = (res[q]["out"] + res[4 + q]["out"] + W12b)
    return out
